# revision 38
# baseline (speedup 1.0000x reference)
"""Local+vertical-strided block-sparse paged attention (decode) on 8 TRN2 cores.

Strategy: tensor-parallel over the 8 KV heads. Core c handles all 16
sequences for its 4 GQA q-heads. The host packs, per (core, seq), EXACTLY
the keys that core's 4 heads can attend to:

  - local window: sparse blocks [qb-15 .. qb]  (<=16 blocks of 64 keys)
  - vertical stride: for each full 512-key group below the window, the
    core's 4 residue blocks (256 keys); plus the needed residue blocks of
    the partial group at the window edge (padded to a parity-uniform count
    so all 8 cores run one identical SPMD program)

The kernel is HBM-DMA bound (~360 GB/s/core), so bytes are minimized with
a split-precision layout validated against the reference:
  - "hot" keys (the <=2 newest window blocks, carrying the most softmax
    weight): float16
  - "cold" keys (older window + all vertical blocks): float8 e3m4
  - mask bias: float8 e5m2 (values are only 0 / -49152)
Measured end-to-end max-rel-error ~1.0e-2 vs the 2e-2 gate.

Host-staged arrays per core (keys are chunked in groups of 128):
  k8 [128, 128*NCH8]  cold K, e3m4, d on partitions
  v8 [128, 128*NCH8]  cold V, e3m4, chunk-local [key, d] blocks
  kH [128, 128*NCHH]  hot K, fp16
  vH [128, 128*NCHH]  hot V, fp16
  bs [128, 4*NCH]     additive mask bias, e5m2
  qT [128, 64]        q, d on partitions, fp16

Device program (sequences sorted largest-first into 5 DMA groups):
  - All DMAs are issued up front on one engine (SP/HWDGE) so transfer
    order == program order: q/bias, all K (smallest group first), then
    all V (smallest group last). The DMA engine never idles mid-kernel.
  - Per group: one score matmul per 128-key chunk into one PSUM bank
    (K chunk stationary, q moving), one group-wide bias add (DVE), one
    Exp (ACT, scale=1/sqrt(128); scores are bounded so no max
    subtraction), denominator matmuls (ones stationary, p moving ->
    [1, 4] per seq; these need only p, not V), then PV matmuls with V
    stationary and p moving (out [d, 4] accumulated in PSUM; output
    free-size 4 keeps tensor-engine time negligible).
  - Normalization constants finish mid-kernel: reciprocal of the [1, 64]
    denominator row, a ones-matmul broadcast to [128, 64], copy to SBUF.
  - Per group, one small DVE multiply scales the PSUM accumulator into
    the fp16 staging tile; a single output DMA ends the kernel, so after
    the final (tiny) V transfer only a few-hundred-ns chain remains.
"""

import numpy as np
import ml_dtypes

NUM_SEQS, MAX_BLOCKS = 16, 256
N_Q_HEADS, N_KV_HEADS, HEAD_SIZE = 32, 8, 128
VLLM_BS, SPARSE_BS = 16, 64
LOCAL_BLOCKS, VERT_STRIDE = 16, 8
MAX_SEQLEN = MAX_BLOCKS * VLLM_BS          # 4096
R = N_Q_HEADS // N_KV_HEADS                # 4
NEG = -49152.0
SM_SCALE = 1.0 / np.sqrt(np.float32(HEAD_SIZE))
BF16 = np.float16
E3M4 = ml_dtypes.float8_e3m4
E5M2 = ml_dtypes.float8_e5m2
HOT_BLOCKS = 2                             # newest window blocks kept in fp16
GROUPS = [4, 4, 4, 3, 1]                   # seqs per DMA group (sorted order)


def _geom(L):
    """Per-sequence packed-layout geometry (identical for every core)."""
    qpos = int(L) - 1
    qb = qpos // SPARSE_BS
    b0 = max(0, qb - (LOCAL_BLOCKS - 1))
    gp, rp = divmod(b0, 8)
    nfull = gp                      # full 512-key groups below the window
    npart = min(rp, 4)              # parity-uniform partial-group block slots
    nwin = qb - b0 + 1              # local-window blocks
    hot = min(nwin, HOT_BLOCKS)
    coldk = 256 * nfull + 64 * npart + 64 * (nwin - hot)
    hotk = 64 * hot
    nch8 = -(-coldk // 128)
    nchh = -(-hotk // 128)
    return dict(qpos=qpos, qb=qb, b0=b0, gp=gp, rp=rp, nfull=nfull,
                npart=npart, nwin=nwin, hot=hot, coldk=coldk, hotk=hotk,
                nch8=nch8, nchh=nchh, nch=nch8 + nchh)


def _keys_for(core, g):
    """Packed cold/hot key lists (within-seq indices) + kind flags.

    kind: 0 = vertical (keep iff residue matches head), 1 = window (keep iff
    causal), 2 = dead filler.
    """
    res = (4, 5, 6, 7) if core % 2 == 0 else (0, 1, 2, 3)
    ck, cf, hk, hf = [], [], [], []

    def blk(keys, flags, b, kd):
        keys.extend(range(b * 64, b * 64 + 64))
        flags.extend([kd] * 64)

    for grp in range(g["nfull"]):
        for r in res:
            blk(ck, cf, grp * 8 + r, 0)
    have = [r for r in res if r < g["rp"]]
    for i in range(g["npart"]):
        if i < len(have):
            blk(ck, cf, g["gp"] * 8 + have[i], 0)
        else:
            blk(ck, cf, g["b0"], 2)
    hot0 = g["qb"] - g["hot"] + 1
    for b in range(g["b0"], hot0):
        blk(ck, cf, b, 1)
    for b in range(hot0, g["qb"] + 1):
        blk(hk, hf, b, 1)

    def pad(keys, flags, n):
        while len(keys) < n:
            keys.append(g["b0"] * 64)
            flags.append(2)
        return (np.asarray(keys[:n], dtype=np.int64),
                np.asarray(flags[:n], dtype=np.int64))

    ck, cf = pad(ck, cf, 128 * g["nch8"])
    hk, hf = pad(hk, hf, 128 * g["nchh"])
    return ck, cf, hk, hf


def _bias_for(core, keys, kind, qpos):
    """[n, 4] additive mask bias in packed order."""
    kb = keys // SPARSE_BS
    h = core * R + np.arange(R)
    vert_keep = (kb[:, None] + h[None, :] + 1) % VERT_STRIDE == 0
    win_keep = (keys <= qpos)[:, None]
    keep = np.where(kind[:, None] == 0, vert_keep,
                    np.where(kind[:, None] == 1, win_keep, False))
    return np.where(keep, np.float32(0.0), np.float32(NEG))


def _layout(context_lens):
    cl = np.asarray(context_lens)
    geos = [_geom(int(cl[s])) for s in range(NUM_SEQS)]
    order = sorted(range(NUM_SEQS), key=lambda s: -geos[s]["nch"])
    return cl, geos, order


def _groups(order):
    out, i = [], 0
    for n in GROUPS:
        out.append(order[i:i + n])
        i += n
    return out


def _pack_v(vsel, nch):
    """[(128*nch), 128] -> [128, nch*128]: chunk-local [key, d] blocks."""
    v3 = vsel.reshape(nch, 128, HEAD_SIZE).transpose(1, 0, 2)
    return np.ascontiguousarray(v3).reshape(HEAD_SIZE, nch * HEAD_SIZE)


def _build_host_arrays(q, k_cache, v_cache, block_tables, context_lens):
    cl, geos, order = _layout(context_lens)
    bt = np.asarray(block_tables)
    n8 = sum(g["nch8"] for g in geos)
    nh = sum(g["nchh"] for g in geos)
    nc_tot = n8 + nh

    q = np.asarray(q, np.float32)
    groups = _groups(order)
    in_maps = []
    for c in range(N_KV_HEADS):
        k8 = np.empty((HEAD_SIZE, 128 * n8), E3M4)
        v8 = np.empty((HEAD_SIZE, 128 * n8), E3M4)
        kH = np.empty((HEAD_SIZE, 128 * nh), BF16)
        vH = np.empty((HEAD_SIZE, 128 * nh), BF16)
        bs = np.empty((HEAD_SIZE, 4 * nc_tot), E5M2)
        o8 = oh = 0
        boff = 0
        for grp in groups:
            for s in grp:
                g = geos[s]
                ck, cf, hk, hf = _keys_for(c, g)
                ks = k_cache[bt[s], c].transpose(1, 0, 2).reshape(
                    HEAD_SIZE, MAX_SEQLEN)
                vs = v_cache[bt[s], c].transpose(0, 2, 1).reshape(
                    MAX_SEQLEN, HEAD_SIZE)
                n8s, nhs, nchs = g["nch8"], g["nchh"], g["nch"]
                k8[:, 128 * o8: 128 * (o8 + n8s)] = ks[:, ck].astype(E3M4)
                v8[:, 128 * o8: 128 * (o8 + n8s)] = _pack_v(vs[ck], n8s).astype(E3M4)
                kH[:, 128 * oh: 128 * (oh + nhs)] = ks[:, hk].astype(BF16)
                vH[:, 128 * oh: 128 * (oh + nhs)] = _pack_v(vs[hk], nhs).astype(BF16)
                bias = np.concatenate(
                    [_bias_for(c, ck, cf, g["qpos"]),
                     _bias_for(c, hk, hf, g["qpos"])], axis=0)   # [128*nch, 4]
                bs[:, boff: boff + 4 * nchs] = (
                    bias.reshape(nchs, 128, R).transpose(1, 0, 2)
                    .reshape(128, R * nchs)).astype(E5M2)
                boff += 4 * nchs
                o8 += n8s
                oh += nhs
        qT = np.ascontiguousarray(
            q[:, c * R:(c + 1) * R, :].transpose(2, 0, 1).reshape(
                HEAD_SIZE, NUM_SEQS * R)).astype(BF16)
        in_maps.append({"k8": k8, "v8": v8, "kH": kH, "vH": vH,
                        "bs": bs, "qT": qT})
    return in_maps, geos, order, n8, nh


def _emulate_core(im, geos, order, n8, nh):
    """Numpy mirror of the device program."""
    k8, v8, kH, vH, bsr, qT = (np.asarray(im[k], np.float32)
                               for k in ("k8", "v8", "kH", "vH", "bs", "qT"))
    out = np.zeros((NUM_SEQS, R, HEAD_SIZE), np.float32)
    o8 = oh = 0
    boff = 0
    for grp in _groups(order):
        for s in grp:
            g = geos[s]
            n8s, nhs, nchs = g["nch8"], g["nchh"], g["nch"]
            kt = np.concatenate(
                [k8[:, 128 * o8: 128 * (o8 + n8s)],
                 kH[:, 128 * oh: 128 * (oh + nhs)]], axis=1)
            bias = bsr[:, boff: boff + 4 * nchs]
            bias = bias.reshape(128, nchs, R).transpose(1, 0, 2).reshape(-1, R)
            scores = kt.T @ qT[:, s * R:(s + 1) * R] + bias
            p = np.exp(SM_SCALE * scores)
            acc = np.zeros((HEAD_SIZE, R), np.float32)
            den = np.zeros((R,), np.float32)
            for i in range(n8s):
                pc = p[128 * i: 128 * (i + 1)]
                acc += v8[:, 128 * (o8 + i): 128 * (o8 + i + 1)].T @ pc
                den += pc.sum(axis=0)
            for i in range(nhs):
                pc = p[128 * (n8s + i): 128 * (n8s + i + 1)]
                acc += vH[:, 128 * (oh + i): 128 * (oh + i + 1)].T @ pc
                den += pc.sum(axis=0)
            out[s] = (acc / den[None, :]).T
            o8 += n8s
            oh += nhs
            boff += 4 * nchs
    return out


def _build_program(geos, order, n8, nh, kv_bufs=None):
    import concourse.bacc as bacc
    import concourse.tile as tile
    from concourse import mybir

    f32 = mybir.dt.float32
    bf16 = mybir.dt.float16
    e3 = mybir.dt.float8e3
    nc = bacc.Bacc("TRN2", target_bir_lowering=False, debug=False, num_devices=8)
    nc_tot = n8 + nh
    NG = len(GROUPS)
    if kv_bufs is None:
        kv_bufs = NG          # all groups resident: DMAs never wait on reuse

    k8D = nc.dram_tensor("k8", [HEAD_SIZE, 128 * n8], e3, kind="ExternalInput")
    v8D = nc.dram_tensor("v8", [HEAD_SIZE, 128 * n8], e3, kind="ExternalInput")
    kHD = nc.dram_tensor("kH", [HEAD_SIZE, 128 * nh], bf16,
                         kind="ExternalInput")
    bsD = nc.dram_tensor("bs", [HEAD_SIZE, 4 * nc_tot], mybir.dt.float8e5,
                         kind="ExternalInput")
    vHD = nc.dram_tensor("vH", [HEAD_SIZE, 128 * nh], bf16, kind="ExternalInput")
    qTD = nc.dram_tensor("qT", [HEAD_SIZE, NUM_SEQS * R], bf16, kind="ExternalInput")
    outD = nc.dram_tensor("out", [HEAD_SIZE, NUM_SEQS * R], bf16,
                          kind="ExternalOutput")

    groups = _groups(order)
    gsz8 = [sum(geos[s]["nch8"] for s in grp) for grp in groups]
    gszh = [sum(geos[s]["nchh"] for s in grp) for grp in groups]
    gszc = [sum(geos[s]["nch"] for s in grp) for grp in groups]
    G8MAX, GHMAX, GCMAX = max(gsz8), max(gszh), max(gszc)
    SR = NUM_SEQS * R

    with tile.TileContext(nc) as tc:
        with (
            tc.tile_pool(name="const", bufs=1) as constp,
            tc.tile_pool(name="k8p", bufs=kv_bufs) as k8p,
            tc.tile_pool(name="v8p", bufs=kv_bufs) as v8p,
            tc.tile_pool(name="khp", bufs=kv_bufs) as khp,
            tc.tile_pool(name="vhp", bufs=kv_bufs) as vhp,
            tc.tile_pool(name="p", bufs=5) as pp,
            tc.tile_pool(name="ps_s", bufs=4, space="PSUM") as ps_s,
            tc.tile_pool(name="ps_o", bufs=2, space="PSUM") as ps_o,
            tc.tile_pool(name="ps_d", bufs=1, space="PSUM") as ps_d,
            tc.tile_pool(name="ps_n", bufs=1, space="PSUM") as ps_n,
        ):
            qt = constp.tile([HEAD_SIZE, NUM_SEQS * R], bf16)
            bs_ = constp.tile([HEAD_SIZE, 4 * nc_tot], mybir.dt.float8e5)
            outacc = constp.tile([HEAD_SIZE, NUM_SEQS * R], bf16)
            rn_sb = constp.tile([HEAD_SIZE, NUM_SEQS * R], f32)
            outtiles = []
            ones_sb = constp.tile([HEAD_SIZE, 1], bf16)
            nc.vector.memset(ones_sb[:], 1.0)
            ones1 = constp.tile([1, HEAD_SIZE], f32)
            nc.vector.memset(ones1[:], 1.0)
            rden_sb = constp.tile([1, SR], f32)
            den_ps = ps_d.tile([1, SR], f32)
            rn_ps = ps_n.tile([HEAD_SIZE, SR], f32)

            # ---- phase 0: issue every DMA up front on ONE engine (SP) so
            # transfer order == program order: q, then all K (smallest group
            # first), then all V (smallest group last). All score/exp work
            # finishes while V still streams; after the final (tiny) V
            # transfer only its PV matmuls + one tiny multiply remain. ----
            NG = len(groups)
            tiles = []
            off8 = [0] * NG
            offh = [0] * NG
            offv = [0] * NG
            o8 = ohh = ohv = 0
            for gi in range(NG):
                off8[gi], offh[gi], offv[gi] = o8, ohh, ohv
                o8 += gsz8[gi]
                ohh += 128 * gszh[gi]
                ohv += gszh[gi]
                k8t = k8p.tile([HEAD_SIZE, 128 * G8MAX], e3, tag="k8")
                v8t = v8p.tile([HEAD_SIZE, 128 * G8MAX], e3, tag="v8")
                kht = khp.tile([HEAD_SIZE, 128 * GHMAX], bf16, tag="kh")
                vht = vhp.tile([HEAD_SIZE, 128 * GHMAX], bf16, tag="vh")
                tiles.append((k8t, v8t, kht, vht))
            first = True
            for gi in [0, NG - 1] + list(range(1, NG - 1)):
                c8, ch, cc = gsz8[gi], gszh[gi], gszc[gi]
                k8t, v8t, kht, vht = tiles[gi]
                if c8:
                    nc.sync.dma_start(
                        k8t[:, :128 * c8],
                        k8D[:, 128 * off8[gi]: 128 * (off8[gi] + c8)])
                if first:
                    # small transfers ride under the big group-0 cold-K one
                    nc.sync.dma_start(qt[:], qTD[:])
                    nc.sync.dma_start(bs_[:], bsD[:])
                    first = False
                nc.sync.dma_start(kht[:, :128 * ch],
                                  kHD[:, offh[gi]: offh[gi] + 128 * ch])
            for gi in list(range(NG - 1)) + [NG - 1]:
                c8, ch = gsz8[gi], gszh[gi]
                k8t, v8t, kht, vht = tiles[gi]
                if c8:
                    nc.sync.dma_start(
                        v8t[:, :128 * c8],
                        v8D[:, 128 * off8[gi]: 128 * (off8[gi] + c8)])
                nc.sync.dma_start(
                    vht[:, :128 * ch],
                    vHD[:, 128 * offv[gi]: 128 * (offv[gi] + ch)])

            # ---- phase 1: per group: scores -> +bias -> Exp -> den -> PV.
            # Denominators need only p (not V), so the whole normalization
            # chain (reciprocal + broadcast matmul + copy to SBUF) completes
            # mid-kernel, while V data is still streaming in. ----
            gstart = [0] * NG
            for gi in range(1, NG):
                gstart[gi] = gstart[gi - 1] + len(groups[gi - 1])
            bsoff = 0
            for gi, grp in enumerate(groups):
                c8, ch, cc = gsz8[gi], gszh[gi], gszc[gi]
                k8t, v8t, kht, vht = tiles[gi]
                sc_ps = ps_s.tile([128, R * GCMAX], f32, tag="sc")
                b8 = bh = bc = 0
                for s in grp:
                    g = geos[s]
                    n8s, nhs = g["nch8"], g["nchh"]
                    for i in range(n8s):
                        nc.tensor.matmul(
                            sc_ps[:, R * (bc + i): R * (bc + i + 1)],
                            k8t[:, 128 * (b8 + i): 128 * (b8 + i + 1)],
                            qt[:, s * R:(s + 1) * R], start=True, stop=True)
                    for i in range(nhs):
                        nc.tensor.matmul(
                            sc_ps[:, R * (bc + n8s + i): R * (bc + n8s + i + 1)],
                            kht[:, 128 * (bh + i): 128 * (bh + i + 1)],
                            qt[:, s * R:(s + 1) * R], start=True, stop=True)
                    b8 += n8s
                    bh += nhs
                    bc += g["nch"]
                nc.vector.tensor_add(
                    sc_ps[:, : R * cc], sc_ps[:, : R * cc],
                    bs_[:, bsoff: bsoff + R * cc])
                bsoff += R * cc
                p_all = pp.tile([128, R * GCMAX], bf16, tag="pall")
                nc.scalar.activation(
                    p_all[:, : R * cc], sc_ps[:, : R * cc],
                    mybir.ActivationFunctionType.Exp, scale=float(SM_SCALE))

                bc = 0
                for t, s in enumerate(grp):
                    tg = gstart[gi] + t
                    nchs = geos[s]["nch"]
                    for i in range(nchs):
                        nc.tensor.matmul(
                            den_ps[:, R * tg: R * (tg + 1)],
                            ones_sb[:],
                            p_all[:, R * (bc + i): R * (bc + i + 1)],
                            start=(i == 0), stop=(i == nchs - 1))
                    bc += nchs

                out_ps = ps_o.tile([HEAD_SIZE, R * len(grp)], f32, tag="ops")
                outtiles.append(out_ps)
                b8 = bh = bc = 0
                for t, s in enumerate(grp):
                    g = geos[s]
                    n8s, nhs = g["nch8"], g["nchh"]
                    for i in range(n8s):
                        nc.tensor.matmul(
                            out_ps[:, R * t: R * (t + 1)],
                            v8t[:, 128 * (b8 + i): 128 * (b8 + i + 1)],
                            p_all[:, R * (bc + i): R * (bc + i + 1)],
                            start=(i == 0), stop=False)
                    for i in range(nhs):
                        nc.tensor.matmul(
                            out_ps[:, R * t: R * (t + 1)],
                            vht[:, 128 * (bh + i): 128 * (bh + i + 1)],
                            p_all[:, R * (bc + n8s + i): R * (bc + n8s + i + 1)],
                            start=(n8s + i == 0), stop=(i == nhs - 1))
                    b8 += n8s
                    bh += nhs
                    bc += g["nch"]

            # ---- phase 2: normalization constants (ready mid-kernel) ----
            nc.vector.reciprocal(rden_sb[:], den_ps[:])
            nc.tensor.matmul(rn_ps[:], ones1[:], rden_sb[:],
                             start=True, stop=True)
            nc.vector.tensor_copy(rn_sb[:], rn_ps[:])
            # per-group: scale PSUM accumulators straight into outacc
            for gi, grp in enumerate(groups):
                ng = len(grp)
                cols = slice(R * gstart[gi], R * (gstart[gi] + ng))
                nc.vector.tensor_mul(outacc[:, cols],
                                     outtiles[gi][:, : R * ng],
                                     rn_sb[:, cols])
            nc.sync.dma_start(outD[:], outacc[:])
    nc.finalize()
    return nc


def kernel(q, k_cache, v_cache, block_tables, context_lens, _emulate=False):
    in_maps, geos, order, n8, nh = _build_host_arrays(
        q, k_cache, v_cache, block_tables, context_lens)

    if _emulate:
        outs = [_emulate_core(in_maps[c], geos, order, n8, nh)
                for c in range(N_KV_HEADS)]
    else:
        import os
        from concourse.bass_utils import run_bass_kernel_spmd
        nc = _build_program(geos, order, n8, nh)
        kw = {}
        if os.environ.get("KERNEL_TRACE"):
            kw = dict(trace=True, trace_cores=list(range(8)),
                      tmpdir=os.environ.get("KERNEL_TRACE_DIR") or None)
        try:
            br = run_bass_kernel_spmd(nc, in_maps, list(range(8)), **kw)
        except Exception:
            # transient device errors (e.g. NRT_EXEC_UNIT_UNRECOVERABLE)
            # clear on re-run
            br = run_bass_kernel_spmd(nc, in_maps, list(range(8)), **kw)
        global LAST_EXEC_NS, LAST_RESULTS
        LAST_RESULTS = br
        LAST_EXEC_NS = br.exec_time_ns
        inv = np.empty(NUM_SEQS, np.int64)
        inv[np.asarray(order)] = np.arange(NUM_SEQS)   # original s -> sorted t
        outs = [np.asarray(br.results[c]["out"]).reshape(
            HEAD_SIZE, NUM_SEQS, R).transpose(1, 2, 0)[inv[np.arange(NUM_SEQS)]]
            for c in range(N_KV_HEADS)]

    out = np.zeros((NUM_SEQS, N_Q_HEADS, HEAD_SIZE), np.float32)
    for c in range(N_KV_HEADS):
        out[:, c * R:(c + 1) * R, :] = outs[c]
    return out


# revision 42
# speedup vs baseline: 1.0165x; 1.0165x over previous
"""Local+vertical-strided block-sparse paged attention (decode) on 8 TRN2 cores.

Strategy: tensor-parallel over the 8 KV heads. Core c handles all 16
sequences for its 4 GQA q-heads. The host packs, per (core, seq), EXACTLY
the keys that core's 4 heads can attend to:

  - local window: sparse blocks [qb-15 .. qb]  (<=16 blocks of 64 keys)
  - vertical stride: for each full 512-key group below the window, the
    core's 4 residue blocks (256 keys); plus the needed residue blocks of
    the partial group at the window edge (padded to a parity-uniform count
    so all 8 cores run one identical SPMD program)

The kernel is HBM-DMA bound (~360 GB/s/core), so bytes are minimized with
a split-precision layout validated against the reference:
  - "hot" keys (the <=2 newest window blocks, carrying the most softmax
    weight): float16
  - "cold" keys (older window + all vertical blocks): float8 e3m4
  - mask bias: float8 e5m2 (values are only 0 / -49152)
Measured end-to-end max-rel-error ~1.0e-2 vs the 2e-2 gate.

Host-staged arrays per core (keys are chunked in groups of 128):
  k8 [128, 128*NCH8]  cold K, e3m4, d on partitions
  v8 [128, 128*NCH8]  cold V, e3m4, chunk-local [key, d] blocks
  kH [128, 128*NCHH]  hot K, fp16
  vH [128, 128*NCHH]  hot V, fp16
  bs [128, 4*NCH]     additive mask bias, e5m2
  qT [128, 64]        q, d on partitions, fp16

Device program (sequences sorted largest-first into 5 DMA groups):
  - All DMAs are issued up front on one engine (SP/HWDGE) so transfer
    order == program order: q/bias, all K (smallest group first), then
    all V (smallest group last). The DMA engine never idles mid-kernel.
  - Per group: one score matmul per 128-key chunk into one PSUM bank
    (K chunk stationary, q moving), one group-wide bias add (DVE), one
    Exp (ACT, scale=1/sqrt(128); scores are bounded so no max
    subtraction), denominator matmuls (ones stationary, p moving ->
    [1, 4] per seq; these need only p, not V), then PV matmuls with V
    stationary and p moving (out [d, 4] accumulated in PSUM; output
    free-size 4 keeps tensor-engine time negligible).
  - Normalization constants finish mid-kernel: reciprocal of the [1, 64]
    denominator row, a ones-matmul broadcast to [128, 64], copy to SBUF.
  - Per group, one small DVE multiply scales the PSUM accumulator into
    the fp16 staging tile; a single output DMA ends the kernel, so after
    the final (tiny) V transfer only a few-hundred-ns chain remains.
"""

import numpy as np
import ml_dtypes

NUM_SEQS, MAX_BLOCKS = 16, 256
N_Q_HEADS, N_KV_HEADS, HEAD_SIZE = 32, 8, 128
VLLM_BS, SPARSE_BS = 16, 64
LOCAL_BLOCKS, VERT_STRIDE = 16, 8
MAX_SEQLEN = MAX_BLOCKS * VLLM_BS          # 4096
R = N_Q_HEADS // N_KV_HEADS                # 4
NEG = -49152.0
SM_SCALE = 1.0 / np.sqrt(np.float32(HEAD_SIZE))
BF16 = np.float16
E3M4 = ml_dtypes.float8_e3m4
E5M2 = ml_dtypes.float8_e5m2
HOT_BLOCKS = 2                             # newest window blocks kept in fp16
GROUPS = [4, 4, 4, 3, 1]                   # seqs per DMA group (sorted order)


def _geom(L):
    """Per-sequence packed-layout geometry (identical for every core)."""
    qpos = int(L) - 1
    qb = qpos // SPARSE_BS
    b0 = max(0, qb - (LOCAL_BLOCKS - 1))
    gp, rp = divmod(b0, 8)
    nfull = gp                      # full 512-key groups below the window
    npart = min(rp, 4)              # parity-uniform partial-group block slots
    nwin = qb - b0 + 1              # local-window blocks
    hot = min(nwin, HOT_BLOCKS)
    coldk = 256 * nfull + 64 * npart + 64 * (nwin - hot)
    hotk = 64 * hot
    hotv = qpos + 1 - 64 * (qb - hot + 1)   # causally-valid hot keys
    nch8 = -(-coldk // 128)
    nchh = -(-hotk // 128)
    return dict(qpos=qpos, qb=qb, b0=b0, gp=gp, rp=rp, nfull=nfull,
                npart=npart, nwin=nwin, hot=hot, coldk=coldk, hotk=hotk,
                hotv=hotv, nch8=nch8, nchh=nchh, nch=nch8 + nchh)


def _keys_for(core, g):
    """Packed cold/hot key lists (within-seq indices) + kind flags.

    kind: 0 = vertical (keep iff residue matches head), 1 = window (keep iff
    causal), 2 = dead filler.
    """
    res = (4, 5, 6, 7) if core % 2 == 0 else (0, 1, 2, 3)
    ck, cf, hk, hf = [], [], [], []

    def blk(keys, flags, b, kd):
        keys.extend(range(b * 64, b * 64 + 64))
        flags.extend([kd] * 64)

    for grp in range(g["nfull"]):
        for r in res:
            blk(ck, cf, grp * 8 + r, 0)
    have = [r for r in res if r < g["rp"]]
    for i in range(g["npart"]):
        if i < len(have):
            blk(ck, cf, g["gp"] * 8 + have[i], 0)
        else:
            blk(ck, cf, g["b0"], 2)
    hot0 = g["qb"] - g["hot"] + 1
    for b in range(g["b0"], hot0):
        blk(ck, cf, b, 1)
    for b in range(hot0, g["qb"] + 1):
        blk(hk, hf, b, 1)

    def pad(keys, flags, n):
        while len(keys) < n:
            keys.append(g["b0"] * 64)
            flags.append(2)
        return (np.asarray(keys[:n], dtype=np.int64),
                np.asarray(flags[:n], dtype=np.int64))

    ck, cf = pad(ck, cf, 128 * g["nch8"])
    hk, hf = pad(hk, hf, 128 * g["nchh"])
    return ck, cf, hk, hf


def _bias_for(core, keys, kind, qpos):
    """[n, 4] additive mask bias in packed order."""
    kb = keys // SPARSE_BS
    h = core * R + np.arange(R)
    vert_keep = (kb[:, None] + h[None, :] + 1) % VERT_STRIDE == 0
    win_keep = (keys <= qpos)[:, None]
    keep = np.where(kind[:, None] == 0, vert_keep,
                    np.where(kind[:, None] == 1, win_keep, False))
    return np.where(keep, np.float32(0.0), np.float32(NEG))


def _layout(context_lens):
    cl = np.asarray(context_lens)
    geos = [_geom(int(cl[s])) for s in range(NUM_SEQS)]
    order = sorted(range(NUM_SEQS), key=lambda s: -geos[s]["nch"])
    return cl, geos, order


def _groups(order):
    out, i = [], 0
    for n in GROUPS:
        out.append(order[i:i + n])
        i += n
    return out


def _pack_v(vsel, nch):
    """[(128*nch), 128] -> [128, nch*128]: chunk-local [key, d] blocks."""
    v3 = vsel.reshape(nch, 128, HEAD_SIZE).transpose(1, 0, 2)
    return np.ascontiguousarray(v3).reshape(HEAD_SIZE, nch * HEAD_SIZE)


def _build_host_arrays(q, k_cache, v_cache, block_tables, context_lens):
    cl, geos, order = _layout(context_lens)
    bt = np.asarray(block_tables)
    n8 = sum(g["nch8"] for g in geos)
    nh = sum(g["nchh"] for g in geos)
    nc_tot = n8 + nh

    q = np.asarray(q, np.float32)
    groups = _groups(order)
    ck_tot = sum(g["coldk"] for g in geos)
    hv_tot = sum(g["hotv"] for g in geos)
    in_maps = []
    for c in range(N_KV_HEADS):
        k8 = np.empty((HEAD_SIZE, ck_tot), E3M4)
        v8 = np.empty((HEAD_SIZE, 128 * n8), E3M4)
        kH = np.empty((HEAD_SIZE, hv_tot), BF16)
        vH = np.empty((HEAD_SIZE, 128 * nh), BF16)
        bs = np.empty((HEAD_SIZE, 4 * nc_tot), E5M2)
        o8 = oh = o8k = ohk = 0
        boff = 0
        for grp in groups:
            for s in grp:
                g = geos[s]
                ck, cf, hk, hf = _keys_for(c, g)
                ks = k_cache[bt[s], c].transpose(1, 0, 2).reshape(
                    HEAD_SIZE, MAX_SEQLEN)
                vs = v_cache[bt[s], c].transpose(0, 2, 1).reshape(
                    MAX_SEQLEN, HEAD_SIZE)
                n8s, nhs, nchs = g["nch8"], g["nchh"], g["nch"]
                ckk, hvk = g["coldk"], g["hotv"]
                k8[:, o8k: o8k + ckk] = ks[:, ck[:ckk]].astype(E3M4)
                v8[:, 128 * o8: 128 * (o8 + n8s)] = _pack_v(vs[ck], n8s).astype(E3M4)
                kH[:, ohk: ohk + hvk] = ks[:, hk[:hvk]].astype(BF16)
                vH[:, 128 * oh: 128 * (oh + nhs)] = _pack_v(vs[hk], nhs).astype(BF16)
                o8k += ckk
                ohk += hvk
                bias = np.concatenate(
                    [_bias_for(c, ck, cf, g["qpos"]),
                     _bias_for(c, hk, hf, g["qpos"])], axis=0)   # [128*nch, 4]
                bs[:, boff: boff + 4 * nchs] = (
                    bias.reshape(nchs, 128, R).transpose(1, 0, 2)
                    .reshape(128, R * nchs)).astype(E5M2)
                boff += 4 * nchs
                o8 += n8s
                oh += nhs
        qT = np.ascontiguousarray(
            q[:, c * R:(c + 1) * R, :].transpose(2, 0, 1).reshape(
                HEAD_SIZE, NUM_SEQS * R)).astype(BF16)
        in_maps.append({"k8": k8, "v8": v8, "kH": kH, "vH": vH,
                        "bs": bs, "qT": qT})
    return in_maps, geos, order, n8, nh


def _emulate_core(im, geos, order, n8, nh):
    """Numpy mirror of the device program."""
    k8, v8, kH, vH, bsr, qT = (np.asarray(im[k], np.float32)
                               for k in ("k8", "v8", "kH", "vH", "bs", "qT"))
    out = np.zeros((NUM_SEQS, R, HEAD_SIZE), np.float32)
    o8 = oh = o8k = ohk = 0
    boff = 0
    for grp in _groups(order):
        for s in grp:
            g = geos[s]
            n8s, nhs, nchs = g["nch8"], g["nchh"], g["nch"]
            ckk, hvk = g["coldk"], g["hotv"]
            kt = np.concatenate(
                [k8[:, o8k: o8k + ckk], kH[:, ohk: ohk + hvk]], axis=1)
            bias_pad = bsr[:, boff: boff + 4 * nchs]
            bias_pad = bias_pad.reshape(128, nchs, R).transpose(
                1, 0, 2)                                # [nch, 128, 4]
            widths = ([min(128, ckk - 128 * i) for i in range(n8s)]
                      + [min(128, hvk - 128 * i) for i in range(nhs)])
            bias = np.concatenate(
                [bias_pad[i, :widths[i]] for i in range(nchs)], axis=0)
            scores = kt.T @ qT[:, s * R:(s + 1) * R] + bias
            p = np.exp(SM_SCALE * scores)
            acc = np.zeros((HEAD_SIZE, R), np.float32)
            den = np.zeros((R,), np.float32)
            row = 0
            for i in range(n8s):
                w = widths[i]
                pc = p[row: row + w]
                acc += v8[:w, 128 * (o8 + i): 128 * (o8 + i) + HEAD_SIZE].T @ pc
                den += pc.sum(axis=0)
                row += w
            for i in range(nhs):
                w = widths[n8s + i]
                pc = p[row: row + w]
                acc += vH[:w, 128 * (oh + i): 128 * (oh + i) + HEAD_SIZE].T @ pc
                den += pc.sum(axis=0)
                row += w
            out[s] = (acc / den[None, :]).T
            o8 += n8s
            oh += nhs
            o8k += ckk
            ohk += hvk
            boff += 4 * nchs
    return out


def _build_program(geos, order, n8, nh, kv_bufs=None):
    import concourse.bacc as bacc
    import concourse.tile as tile
    from concourse import mybir

    f32 = mybir.dt.float32
    bf16 = mybir.dt.float16
    e3 = mybir.dt.float8e3
    nc = bacc.Bacc("TRN2", target_bir_lowering=False, debug=False, num_devices=8)
    nc_tot = n8 + nh
    NG = len(GROUPS)
    if kv_bufs is None:
        kv_bufs = NG          # all groups resident: DMAs never wait on reuse

    ck_tot = sum(g["coldk"] for g in geos)
    hv_tot = sum(g["hotv"] for g in geos)
    k8D = nc.dram_tensor("k8", [HEAD_SIZE, ck_tot], e3, kind="ExternalInput")
    v8D = nc.dram_tensor("v8", [HEAD_SIZE, 128 * n8], e3, kind="ExternalInput")
    kHD = nc.dram_tensor("kH", [HEAD_SIZE, hv_tot], bf16,
                         kind="ExternalInput")
    bsD = nc.dram_tensor("bs", [HEAD_SIZE, 4 * nc_tot], mybir.dt.float8e5,
                         kind="ExternalInput")
    vHD = nc.dram_tensor("vH", [HEAD_SIZE, 128 * nh], bf16, kind="ExternalInput")
    qTD = nc.dram_tensor("qT", [HEAD_SIZE, NUM_SEQS * R], bf16, kind="ExternalInput")
    outD = nc.dram_tensor("out", [HEAD_SIZE, NUM_SEQS * R], bf16,
                          kind="ExternalOutput")

    groups = _groups(order)
    gsz8 = [sum(geos[s]["nch8"] for s in grp) for grp in groups]
    gszh = [sum(geos[s]["nchh"] for s in grp) for grp in groups]
    gszc = [sum(geos[s]["nch"] for s in grp) for grp in groups]
    gk8 = [sum(geos[s]["coldk"] for s in grp) for grp in groups]
    gkh = [sum(geos[s]["hotv"] for s in grp) for grp in groups]
    G8MAX, GHMAX, GCMAX = max(gsz8), max(gszh), max(gszc)
    SR = NUM_SEQS * R

    with tile.TileContext(nc) as tc:
        with (
            tc.tile_pool(name="const", bufs=1) as constp,
            tc.tile_pool(name="k8p", bufs=kv_bufs) as k8p,
            tc.tile_pool(name="v8p", bufs=kv_bufs) as v8p,
            tc.tile_pool(name="khp", bufs=kv_bufs) as khp,
            tc.tile_pool(name="vhp", bufs=kv_bufs) as vhp,
            tc.tile_pool(name="p", bufs=5) as pp,
            tc.tile_pool(name="ps_s", bufs=4, space="PSUM") as ps_s,
            tc.tile_pool(name="ps_o", bufs=2, space="PSUM") as ps_o,
            tc.tile_pool(name="ps_d", bufs=1, space="PSUM") as ps_d,
            tc.tile_pool(name="ps_n", bufs=1, space="PSUM") as ps_n,
        ):
            qt = constp.tile([HEAD_SIZE, NUM_SEQS * R], bf16)
            bs_ = constp.tile([HEAD_SIZE, 4 * nc_tot], mybir.dt.float8e5)
            outacc = constp.tile([HEAD_SIZE, NUM_SEQS * R], bf16)
            rn_sb = constp.tile([HEAD_SIZE, NUM_SEQS * R], f32)
            outtiles = []
            ones_sb = constp.tile([HEAD_SIZE, 1], bf16)
            nc.vector.memset(ones_sb[:], 1.0)
            ones1 = constp.tile([1, HEAD_SIZE], f32)
            nc.vector.memset(ones1[:], 1.0)
            rden_sb = constp.tile([1, SR], f32)
            den_ps = ps_d.tile([1, SR], f32)
            rn_ps = ps_n.tile([HEAD_SIZE, SR], f32)

            # ---- phase 0: issue every DMA up front on ONE engine (SP) so
            # transfer order == program order: q, then all K (smallest group
            # first), then all V (smallest group last). All score/exp work
            # finishes while V still streams; after the final (tiny) V
            # transfer only its PV matmuls + one tiny multiply remain. ----
            NG = len(groups)
            tiles = []
            off8 = [0] * NG
            offh = [0] * NG
            offv = [0] * NG
            o8 = ohh = ohv = 0
            for gi in range(NG):
                off8[gi], offh[gi], offv[gi] = o8, ohh, ohv
                o8 += gk8[gi]
                ohh += gkh[gi]
                ohv += gszh[gi]
                k8t = k8p.tile([HEAD_SIZE, 128 * G8MAX], e3, tag="k8")
                v8t = v8p.tile([HEAD_SIZE, 128 * G8MAX], e3, tag="v8")
                kht = khp.tile([HEAD_SIZE, 128 * GHMAX], bf16, tag="kh")
                vht = vhp.tile([HEAD_SIZE, 128 * GHMAX], bf16, tag="vh")
                tiles.append((k8t, v8t, kht, vht))
            first = True
            for gi in [0, NG - 1] + list(range(1, NG - 1)):
                k8t, v8t, kht, vht = tiles[gi]
                if gk8[gi]:
                    nc.sync.dma_start(
                        k8t[:, :gk8[gi]],
                        k8D[:, off8[gi]: off8[gi] + gk8[gi]])
                if first:
                    # small transfers ride under the big group-0 cold-K one
                    nc.sync.dma_start(qt[:], qTD[:])
                    nc.sync.dma_start(bs_[:], bsD[:])
                    first = False
                nc.sync.dma_start(kht[:, :gkh[gi]],
                                  kHD[:, offh[gi]: offh[gi] + gkh[gi]])
            offv8 = [0] * NG
            a = 0
            for gi in range(NG):
                offv8[gi] = a
                a += gsz8[gi]
            for gi in list(range(NG - 1)) + [NG - 1]:
                c8, ch = gsz8[gi], gszh[gi]
                k8t, v8t, kht, vht = tiles[gi]
                if c8:
                    nc.sync.dma_start(
                        v8t[:, :128 * c8],
                        v8D[:, 128 * offv8[gi]: 128 * (offv8[gi] + c8)])
                nc.sync.dma_start(
                    vht[:, :128 * ch],
                    vHD[:, 128 * offv[gi]: 128 * (offv[gi] + ch)])

            # ---- phase 1: per group: scores -> +bias -> Exp -> den -> PV.
            # Denominators need only p (not V), so the whole normalization
            # chain (reciprocal + broadcast matmul + copy to SBUF) completes
            # mid-kernel, while V data is still streaming in. ----
            gstart = [0] * NG
            for gi in range(1, NG):
                gstart[gi] = gstart[gi - 1] + len(groups[gi - 1])
            bsoff = 0
            for gi, grp in enumerate(groups):
                c8, ch, cc = gsz8[gi], gszh[gi], gszc[gi]
                k8t, v8t, kht, vht = tiles[gi]
                sc_ps = ps_s.tile([128, R * GCMAX], f32, tag="sc")
                b8 = bh = bc = 0     # K column offsets inside the group tiles
                for s in grp:
                    g = geos[s]
                    n8s, nhs = g["nch8"], g["nchh"]
                    ckk, hvk = g["coldk"], g["hotv"]
                    for i in range(n8s):
                        w = min(128, ckk - 128 * i)
                        nc.tensor.matmul(
                            sc_ps[0:w, R * (bc + i): R * (bc + i + 1)],
                            k8t[:, b8 + 128 * i: b8 + 128 * i + w],
                            qt[:, s * R:(s + 1) * R], start=True, stop=True)
                    for i in range(nhs):
                        w = min(128, hvk - 128 * i)
                        nc.tensor.matmul(
                            sc_ps[0:w, R * (bc + n8s + i): R * (bc + n8s + i + 1)],
                            kht[:, bh + 128 * i: bh + 128 * i + w],
                            qt[:, s * R:(s + 1) * R], start=True, stop=True)
                    b8 += ckk
                    bh += hvk
                    bc += g["nch"]
                nc.vector.tensor_add(
                    sc_ps[:, : R * cc], sc_ps[:, : R * cc],
                    bs_[:, bsoff: bsoff + R * cc])
                bsoff += R * cc
                p_all = pp.tile([128, R * GCMAX], bf16, tag="pall")
                nc.scalar.activation(
                    p_all[:, : R * cc], sc_ps[:, : R * cc],
                    mybir.ActivationFunctionType.Exp, scale=float(SM_SCALE))

                bc = 0
                for t, s in enumerate(grp):
                    tg = gstart[gi] + t
                    g = geos[s]
                    n8s, nhs = g["nch8"], g["nchh"]
                    nchs = g["nch"]
                    widths = ([min(128, g["coldk"] - 128 * i) for i in range(n8s)]
                              + [min(128, g["hotv"] - 128 * i) for i in range(nhs)])
                    for i in range(nchs):
                        w = widths[i]
                        nc.tensor.matmul(
                            den_ps[:, R * tg: R * (tg + 1)],
                            ones_sb[0:w, :],
                            p_all[0:w, R * (bc + i): R * (bc + i + 1)],
                            start=(i == 0), stop=(i == nchs - 1))
                    bc += nchs

                out_ps = ps_o.tile([HEAD_SIZE, R * len(grp)], f32, tag="ops")
                outtiles.append(out_ps)
                b8 = bh = bc = 0
                for t, s in enumerate(grp):
                    g = geos[s]
                    n8s, nhs = g["nch8"], g["nchh"]
                    for i in range(n8s):
                        w = min(128, g["coldk"] - 128 * i)
                        nc.tensor.matmul(
                            out_ps[:, R * t: R * (t + 1)],
                            v8t[0:w, 128 * (b8 + i): 128 * (b8 + i) + HEAD_SIZE],
                            p_all[0:w, R * (bc + i): R * (bc + i + 1)],
                            start=(i == 0), stop=False)
                    for i in range(nhs):
                        w = min(128, g["hotv"] - 128 * i)
                        nc.tensor.matmul(
                            out_ps[:, R * t: R * (t + 1)],
                            vht[0:w, 128 * (bh + i): 128 * (bh + i) + HEAD_SIZE],
                            p_all[0:w, R * (bc + n8s + i): R * (bc + n8s + i + 1)],
                            start=(n8s + i == 0), stop=(i == nhs - 1))
                    b8 += n8s
                    bh += nhs
                    bc += g["nch"]

            # ---- phase 2: normalization constants (ready mid-kernel) ----
            nc.vector.reciprocal(rden_sb[:], den_ps[:])
            nc.tensor.matmul(rn_ps[:], ones1[:], rden_sb[:],
                             start=True, stop=True)
            nc.vector.tensor_copy(rn_sb[:], rn_ps[:])
            # per-group: scale PSUM accumulators straight into outacc
            for gi, grp in enumerate(groups):
                ng = len(grp)
                cols = slice(R * gstart[gi], R * (gstart[gi] + ng))
                nc.vector.tensor_mul(outacc[:, cols],
                                     outtiles[gi][:, : R * ng],
                                     rn_sb[:, cols])
            nc.sync.dma_start(outD[:], outacc[:])
    nc.finalize()
    return nc


def kernel(q, k_cache, v_cache, block_tables, context_lens, _emulate=False):
    in_maps, geos, order, n8, nh = _build_host_arrays(
        q, k_cache, v_cache, block_tables, context_lens)

    if _emulate:
        outs = [_emulate_core(in_maps[c], geos, order, n8, nh)
                for c in range(N_KV_HEADS)]
    else:
        import os
        from concourse.bass_utils import run_bass_kernel_spmd
        nc = _build_program(geos, order, n8, nh)
        kw = {}
        if os.environ.get("KERNEL_TRACE"):
            kw = dict(trace=True, trace_cores=list(range(8)),
                      tmpdir=os.environ.get("KERNEL_TRACE_DIR") or None)
        try:
            br = run_bass_kernel_spmd(nc, in_maps, list(range(8)), **kw)
        except Exception:
            # transient device errors (e.g. NRT_EXEC_UNIT_UNRECOVERABLE)
            # clear on re-run
            br = run_bass_kernel_spmd(nc, in_maps, list(range(8)), **kw)
        global LAST_EXEC_NS, LAST_RESULTS
        LAST_RESULTS = br
        LAST_EXEC_NS = br.exec_time_ns
        inv = np.empty(NUM_SEQS, np.int64)
        inv[np.asarray(order)] = np.arange(NUM_SEQS)   # original s -> sorted t
        outs = [np.asarray(br.results[c]["out"]).reshape(
            HEAD_SIZE, NUM_SEQS, R).transpose(1, 2, 0)[inv[np.arange(NUM_SEQS)]]
            for c in range(N_KV_HEADS)]

    out = np.zeros((NUM_SEQS, N_Q_HEADS, HEAD_SIZE), np.float32)
    for c in range(N_KV_HEADS):
        out[:, c * R:(c + 1) * R, :] = outs[c]
    return out


# revision 44
# speedup vs baseline: 1.0178x; 1.0012x over previous
"""Local+vertical-strided block-sparse paged attention (decode) on 8 TRN2 cores.

Strategy: tensor-parallel over the 8 KV heads. Core c handles all 16
sequences for its 4 GQA q-heads. The host packs, per (core, seq), EXACTLY
the keys that core's 4 heads can attend to:

  - local window: sparse blocks [qb-15 .. qb]  (<=16 blocks of 64 keys)
  - vertical stride: for each full 512-key group below the window, the
    core's 4 residue blocks (256 keys); plus the needed residue blocks of
    the partial group at the window edge (padded to a parity-uniform count
    so all 8 cores run one identical SPMD program)

The kernel is HBM-DMA bound (~360 GB/s/core), so bytes are minimized with
a split-precision layout validated against the reference:
  - "hot" keys (the <=2 newest window blocks, carrying the most softmax
    weight): float16
  - "cold" keys (older window + all vertical blocks): float8 e3m4
  - mask bias: float8 e5m2 (values are only 0 / -49152)
Measured end-to-end max-rel-error ~1.0e-2 vs the 2e-2 gate.

Host-staged arrays per core (keys are chunked in groups of 128):
  k8 [128, 128*NCH8]  cold K, e3m4, d on partitions
  v8 [128, 128*NCH8]  cold V, e3m4, chunk-local [key, d] blocks
  kH [128, 128*NCHH]  hot K, fp16
  vH [128, 128*NCHH]  hot V, fp16
  bs [128, 4*NCH]     additive mask bias, e5m2
  qT [128, 64]        q, d on partitions, fp16

Device program (sequences sorted largest-first into 5 DMA groups):
  - All DMAs are issued up front on one engine (SP/HWDGE) so transfer
    order == program order: q/bias, all K (smallest group first), then
    all V (smallest group last). The DMA engine never idles mid-kernel.
  - Per group: one score matmul per 128-key chunk into one PSUM bank
    (K chunk stationary, q moving), one group-wide bias add (DVE), one
    Exp (ACT, scale=1/sqrt(128); scores are bounded so no max
    subtraction), denominator matmuls (ones stationary, p moving ->
    [1, 4] per seq; these need only p, not V), then PV matmuls with V
    stationary and p moving (out [d, 4] accumulated in PSUM; output
    free-size 4 keeps tensor-engine time negligible).
  - Normalization constants finish mid-kernel: reciprocal of the [1, 64]
    denominator row, a ones-matmul broadcast to [128, 64], copy to SBUF.
  - Per group, one small DVE multiply scales the PSUM accumulator into
    the fp16 staging tile; a single output DMA ends the kernel, so after
    the final (tiny) V transfer only a few-hundred-ns chain remains.
"""

import numpy as np
import ml_dtypes

NUM_SEQS, MAX_BLOCKS = 16, 256
N_Q_HEADS, N_KV_HEADS, HEAD_SIZE = 32, 8, 128
VLLM_BS, SPARSE_BS = 16, 64
LOCAL_BLOCKS, VERT_STRIDE = 16, 8
MAX_SEQLEN = MAX_BLOCKS * VLLM_BS          # 4096
R = N_Q_HEADS // N_KV_HEADS                # 4
NEG = -49152.0
SM_SCALE = 1.0 / np.sqrt(np.float32(HEAD_SIZE))
BF16 = np.float16
E3M4 = ml_dtypes.float8_e3m4
E5M2 = ml_dtypes.float8_e5m2
HOT_BLOCKS = 2                             # newest window blocks kept in fp16
GROUPS = [4, 4, 4, 3, 1]                   # seqs per DMA group (sorted order)


def _geom(L):
    """Per-sequence packed-layout geometry (identical for every core)."""
    qpos = int(L) - 1
    qb = qpos // SPARSE_BS
    b0 = max(0, qb - (LOCAL_BLOCKS - 1))
    gp, rp = divmod(b0, 8)
    nfull = gp                      # full 512-key groups below the window
    npart = min(rp, 4)              # parity-uniform partial-group block slots
    nwin = qb - b0 + 1              # local-window blocks
    hot = min(nwin, HOT_BLOCKS)
    coldk = 256 * nfull + 64 * npart + 64 * (nwin - hot)
    hotk = 64 * hot
    hotv = qpos + 1 - 64 * (qb - hot + 1)   # causally-valid hot keys
    nch8 = -(-coldk // 128)
    nchh = -(-hotk // 128)
    vk = 256 * nfull + 64 * npart           # vertical-region keys
    nbch = -(-vk // 128)                    # chunks needing a mask bias
    return dict(qpos=qpos, qb=qb, b0=b0, gp=gp, rp=rp, nfull=nfull,
                npart=npart, nwin=nwin, hot=hot, coldk=coldk, hotk=hotk,
                hotv=hotv, nch8=nch8, nchh=nchh, nch=nch8 + nchh, nbch=nbch)


def _keys_for(core, g):
    """Packed cold/hot key lists (within-seq indices) + kind flags.

    kind: 0 = vertical (keep iff residue matches head), 1 = window (keep iff
    causal), 2 = dead filler.
    """
    res = (4, 5, 6, 7) if core % 2 == 0 else (0, 1, 2, 3)
    ck, cf, hk, hf = [], [], [], []

    def blk(keys, flags, b, kd):
        keys.extend(range(b * 64, b * 64 + 64))
        flags.extend([kd] * 64)

    for grp in range(g["nfull"]):
        for r in res:
            blk(ck, cf, grp * 8 + r, 0)
    have = [r for r in res if r < g["rp"]]
    for i in range(g["npart"]):
        if i < len(have):
            blk(ck, cf, g["gp"] * 8 + have[i], 0)
        else:
            blk(ck, cf, g["b0"], 2)
    hot0 = g["qb"] - g["hot"] + 1
    for b in range(g["b0"], hot0):
        blk(ck, cf, b, 1)
    for b in range(hot0, g["qb"] + 1):
        blk(hk, hf, b, 1)

    def pad(keys, flags, n):
        while len(keys) < n:
            keys.append(g["b0"] * 64)
            flags.append(2)
        return (np.asarray(keys[:n], dtype=np.int64),
                np.asarray(flags[:n], dtype=np.int64))

    ck, cf = pad(ck, cf, 128 * g["nch8"])
    hk, hf = pad(hk, hf, 128 * g["nchh"])
    return ck, cf, hk, hf


def _bias_for(core, keys, kind, qpos):
    """[n, 4] additive mask bias in packed order."""
    kb = keys // SPARSE_BS
    h = core * R + np.arange(R)
    vert_keep = (kb[:, None] + h[None, :] + 1) % VERT_STRIDE == 0
    win_keep = (keys <= qpos)[:, None]
    keep = np.where(kind[:, None] == 0, vert_keep,
                    np.where(kind[:, None] == 1, win_keep, False))
    return np.where(keep, np.float32(0.0), np.float32(NEG))


def _layout(context_lens):
    cl = np.asarray(context_lens)
    geos = [_geom(int(cl[s])) for s in range(NUM_SEQS)]
    order = sorted(range(NUM_SEQS), key=lambda s: -geos[s]["nch"])
    return cl, geos, order


def _groups(order):
    out, i = [], 0
    for n in GROUPS:
        out.append(order[i:i + n])
        i += n
    return out


def _pack_v(vsel, nch):
    """[(128*nch), 128] -> [128, nch*128]: chunk-local [key, d] blocks."""
    v3 = vsel.reshape(nch, 128, HEAD_SIZE).transpose(1, 0, 2)
    return np.ascontiguousarray(v3).reshape(HEAD_SIZE, nch * HEAD_SIZE)


def _build_host_arrays(q, k_cache, v_cache, block_tables, context_lens):
    cl, geos, order = _layout(context_lens)
    bt = np.asarray(block_tables)
    n8 = sum(g["nch8"] for g in geos)
    nh = sum(g["nchh"] for g in geos)
    nc_tot = sum(g["nbch"] for g in geos)

    q = np.asarray(q, np.float32)
    groups = _groups(order)
    ck_tot = sum(g["coldk"] for g in geos)
    hv_tot = sum(g["hotv"] for g in geos)
    in_maps = []
    for c in range(N_KV_HEADS):
        k8 = np.empty((HEAD_SIZE, ck_tot), E3M4)
        v8 = np.empty((HEAD_SIZE, 128 * n8), E3M4)
        kH = np.empty((HEAD_SIZE, hv_tot), BF16)
        vH = np.empty((HEAD_SIZE, 128 * nh), BF16)
        bs = np.empty((HEAD_SIZE, 4 * nc_tot), E5M2)
        o8 = oh = o8k = ohk = 0
        boff = 0
        for grp in groups:
            for s in grp:
                g = geos[s]
                ck, cf, hk, hf = _keys_for(c, g)
                ks = k_cache[bt[s], c].transpose(1, 0, 2).reshape(
                    HEAD_SIZE, MAX_SEQLEN)
                vs = v_cache[bt[s], c].transpose(0, 2, 1).reshape(
                    MAX_SEQLEN, HEAD_SIZE)
                n8s, nhs, nchs = g["nch8"], g["nchh"], g["nch"]
                ckk, hvk = g["coldk"], g["hotv"]
                k8[:, o8k: o8k + ckk] = ks[:, ck[:ckk]].astype(E3M4)
                v8[:, 128 * o8: 128 * (o8 + n8s)] = _pack_v(vs[ck], n8s).astype(E3M4)
                kH[:, ohk: ohk + hvk] = ks[:, hk[:hvk]].astype(BF16)
                vH[:, 128 * oh: 128 * (oh + nhs)] = _pack_v(vs[hk], nhs).astype(BF16)
                o8k += ckk
                ohk += hvk
                nb = g["nbch"]
                if nb:
                    bias = _bias_for(c, ck[:128 * nb], cf[:128 * nb],
                                     g["qpos"])              # [128*nb, 4]
                    bs[:, boff: boff + 4 * nb] = (
                        bias.reshape(nb, 128, R).transpose(1, 0, 2)
                        .reshape(128, R * nb)).astype(E5M2)
                boff += 4 * nb
                o8 += n8s
                oh += nhs
        qT = np.ascontiguousarray(
            q[:, c * R:(c + 1) * R, :].transpose(2, 0, 1).reshape(
                HEAD_SIZE, NUM_SEQS * R)).astype(BF16)
        in_maps.append({"k8": k8, "v8": v8, "kH": kH, "vH": vH,
                        "bs": bs, "qT": qT})
    return in_maps, geos, order, n8, nh


def _emulate_core(im, geos, order, n8, nh):
    """Numpy mirror of the device program."""
    k8, v8, kH, vH, bsr, qT = (np.asarray(im[k], np.float32)
                               for k in ("k8", "v8", "kH", "vH", "bs", "qT"))
    out = np.zeros((NUM_SEQS, R, HEAD_SIZE), np.float32)
    o8 = oh = o8k = ohk = 0
    boff = 0
    for grp in _groups(order):
        for s in grp:
            g = geos[s]
            n8s, nhs, nchs = g["nch8"], g["nchh"], g["nch"]
            ckk, hvk, nb = g["coldk"], g["hotv"], g["nbch"]
            kt = np.concatenate(
                [k8[:, o8k: o8k + ckk], kH[:, ohk: ohk + hvk]], axis=1)
            widths = ([min(128, ckk - 128 * i) for i in range(n8s)]
                      + [min(128, hvk - 128 * i) for i in range(nhs)])
            bias = np.zeros((ckk + hvk, R), np.float32)
            if nb:
                bias_pad = bsr[:, boff: boff + 4 * nb]
                bias_pad = bias_pad.reshape(128, nb, R).transpose(1, 0, 2)
                bb = bias_pad.reshape(128 * nb, R)
                n = min(128 * nb, ckk)
                bias[:n] = bb[:n]
            scores = kt.T @ qT[:, s * R:(s + 1) * R] + bias
            p = np.exp(SM_SCALE * scores)
            acc = np.zeros((HEAD_SIZE, R), np.float32)
            den = np.zeros((R,), np.float32)
            row = 0
            for i in range(n8s):
                w = widths[i]
                pc = p[row: row + w]
                acc += v8[:w, 128 * (o8 + i): 128 * (o8 + i) + HEAD_SIZE].T @ pc
                den += pc.sum(axis=0)
                row += w
            for i in range(nhs):
                w = widths[n8s + i]
                pc = p[row: row + w]
                acc += vH[:w, 128 * (oh + i): 128 * (oh + i) + HEAD_SIZE].T @ pc
                den += pc.sum(axis=0)
                row += w
            out[s] = (acc / den[None, :]).T
            o8 += n8s
            oh += nhs
            o8k += ckk
            ohk += hvk
            boff += 4 * nb
    return out


def _build_program(geos, order, n8, nh, kv_bufs=None):
    import concourse.bacc as bacc
    import concourse.tile as tile
    from concourse import mybir

    f32 = mybir.dt.float32
    bf16 = mybir.dt.float16
    e3 = mybir.dt.float8e3
    nc = bacc.Bacc("TRN2", target_bir_lowering=False, debug=False, num_devices=8)
    nc_tot = sum(g["nbch"] for g in geos)
    NG = len(GROUPS)
    if kv_bufs is None:
        kv_bufs = NG          # all groups resident: DMAs never wait on reuse

    ck_tot = sum(g["coldk"] for g in geos)
    hv_tot = sum(g["hotv"] for g in geos)
    k8D = nc.dram_tensor("k8", [HEAD_SIZE, ck_tot], e3, kind="ExternalInput")
    v8D = nc.dram_tensor("v8", [HEAD_SIZE, 128 * n8], e3, kind="ExternalInput")
    kHD = nc.dram_tensor("kH", [HEAD_SIZE, hv_tot], bf16,
                         kind="ExternalInput")
    bsD = nc.dram_tensor("bs", [HEAD_SIZE, 4 * nc_tot], mybir.dt.float8e5,
                         kind="ExternalInput")
    vHD = nc.dram_tensor("vH", [HEAD_SIZE, 128 * nh], bf16, kind="ExternalInput")
    qTD = nc.dram_tensor("qT", [HEAD_SIZE, NUM_SEQS * R], bf16, kind="ExternalInput")
    outD = nc.dram_tensor("out", [HEAD_SIZE, NUM_SEQS * R], bf16,
                          kind="ExternalOutput")

    groups = _groups(order)
    gsz8 = [sum(geos[s]["nch8"] for s in grp) for grp in groups]
    gszh = [sum(geos[s]["nchh"] for s in grp) for grp in groups]
    gszc = [sum(geos[s]["nch"] for s in grp) for grp in groups]
    gk8 = [sum(geos[s]["coldk"] for s in grp) for grp in groups]
    gkh = [sum(geos[s]["hotv"] for s in grp) for grp in groups]
    G8MAX, GHMAX, GCMAX = max(gsz8), max(gszh), max(gszc)
    SR = NUM_SEQS * R

    with tile.TileContext(nc) as tc:
        with (
            tc.tile_pool(name="const", bufs=1) as constp,
            tc.tile_pool(name="k8p", bufs=kv_bufs) as k8p,
            tc.tile_pool(name="v8p", bufs=kv_bufs) as v8p,
            tc.tile_pool(name="khp", bufs=kv_bufs) as khp,
            tc.tile_pool(name="vhp", bufs=kv_bufs) as vhp,
            tc.tile_pool(name="p", bufs=5) as pp,
            tc.tile_pool(name="ps_s", bufs=4, space="PSUM") as ps_s,
            tc.tile_pool(name="ps_o", bufs=2, space="PSUM") as ps_o,
            tc.tile_pool(name="ps_d", bufs=1, space="PSUM") as ps_d,
            tc.tile_pool(name="ps_n", bufs=1, space="PSUM") as ps_n,
        ):
            qt = constp.tile([HEAD_SIZE, NUM_SEQS * R], bf16)
            bs_ = constp.tile([HEAD_SIZE, 4 * nc_tot], mybir.dt.float8e5)
            outacc = constp.tile([HEAD_SIZE, NUM_SEQS * R], bf16)
            rn_sb = constp.tile([HEAD_SIZE, NUM_SEQS * R], f32)
            outtiles = []
            ones_sb = constp.tile([HEAD_SIZE, 1], bf16)
            nc.vector.memset(ones_sb[:], 1.0)
            ones1 = constp.tile([1, HEAD_SIZE], f32)
            nc.vector.memset(ones1[:], 1.0)
            rden_sb = constp.tile([1, SR], f32)
            den_ps = ps_d.tile([1, SR], f32)
            rn_ps = ps_n.tile([HEAD_SIZE, SR], f32)

            # ---- phase 0: issue every DMA up front on ONE engine (SP) so
            # transfer order == program order: q, then all K (smallest group
            # first), then all V (smallest group last). All score/exp work
            # finishes while V still streams; after the final (tiny) V
            # transfer only its PV matmuls + one tiny multiply remain. ----
            NG = len(groups)
            tiles = []
            off8 = [0] * NG
            offh = [0] * NG
            offv = [0] * NG
            o8 = ohh = ohv = 0
            for gi in range(NG):
                off8[gi], offh[gi], offv[gi] = o8, ohh, ohv
                o8 += gk8[gi]
                ohh += gkh[gi]
                ohv += gszh[gi]
                k8t = k8p.tile([HEAD_SIZE, 128 * G8MAX], e3, tag="k8")
                v8t = v8p.tile([HEAD_SIZE, 128 * G8MAX], e3, tag="v8")
                kht = khp.tile([HEAD_SIZE, 128 * GHMAX], bf16, tag="kh")
                vht = vhp.tile([HEAD_SIZE, 128 * GHMAX], bf16, tag="vh")
                tiles.append((k8t, v8t, kht, vht))
            first = True
            for gi in [0, NG - 1] + list(range(1, NG - 1)):
                k8t, v8t, kht, vht = tiles[gi]
                if gk8[gi]:
                    nc.sync.dma_start(
                        k8t[:, :gk8[gi]],
                        k8D[:, off8[gi]: off8[gi] + gk8[gi]])
                if first:
                    # small transfers ride under the big group-0 cold-K one
                    nc.sync.dma_start(qt[:], qTD[:])
                    nc.sync.dma_start(bs_[:], bsD[:])
                    first = False
                nc.sync.dma_start(kht[:, :gkh[gi]],
                                  kHD[:, offh[gi]: offh[gi] + gkh[gi]])
            offv8 = [0] * NG
            a = 0
            for gi in range(NG):
                offv8[gi] = a
                a += gsz8[gi]
            for gi in list(range(NG - 1)) + [NG - 1]:
                c8, ch = gsz8[gi], gszh[gi]
                k8t, v8t, kht, vht = tiles[gi]
                if c8:
                    nc.sync.dma_start(
                        v8t[:, :128 * c8],
                        v8D[:, 128 * offv8[gi]: 128 * (offv8[gi] + c8)])
                nc.sync.dma_start(
                    vht[:, :128 * ch],
                    vHD[:, 128 * offv[gi]: 128 * (offv[gi] + ch)])

            # ---- phase 1: per group: scores -> +bias -> Exp -> den -> PV.
            # Denominators need only p (not V), so the whole normalization
            # chain (reciprocal + broadcast matmul + copy to SBUF) completes
            # mid-kernel, while V data is still streaming in. ----
            gstart = [0] * NG
            for gi in range(1, NG):
                gstart[gi] = gstart[gi - 1] + len(groups[gi - 1])
            bsoff = 0
            for gi, grp in enumerate(groups):
                c8, ch, cc = gsz8[gi], gszh[gi], gszc[gi]
                k8t, v8t, kht, vht = tiles[gi]
                sc_ps = ps_s.tile([128, R * GCMAX], f32, tag="sc")
                b8 = bh = bc = 0     # K column offsets inside the group tiles
                for s in grp:
                    g = geos[s]
                    n8s, nhs = g["nch8"], g["nchh"]
                    ckk, hvk = g["coldk"], g["hotv"]
                    for i in range(n8s):
                        w = min(128, ckk - 128 * i)
                        nc.tensor.matmul(
                            sc_ps[0:w, R * (bc + i): R * (bc + i + 1)],
                            k8t[:, b8 + 128 * i: b8 + 128 * i + w],
                            qt[:, s * R:(s + 1) * R], start=True, stop=True)
                    for i in range(nhs):
                        w = min(128, hvk - 128 * i)
                        nc.tensor.matmul(
                            sc_ps[0:w, R * (bc + n8s + i): R * (bc + n8s + i + 1)],
                            kht[:, bh + 128 * i: bh + 128 * i + w],
                            qt[:, s * R:(s + 1) * R], start=True, stop=True)
                    b8 += ckk
                    bh += hvk
                    bc += g["nch"]
                bc2 = 0
                for s in grp:
                    nb = geos[s]["nbch"]
                    if nb:
                        nc.vector.tensor_add(
                            sc_ps[:, R * bc2: R * (bc2 + nb)],
                            sc_ps[:, R * bc2: R * (bc2 + nb)],
                            bs_[:, bsoff: bsoff + R * nb])
                    bsoff += R * nb
                    bc2 += geos[s]["nch"]
                p_all = pp.tile([128, R * GCMAX], bf16, tag="pall")
                nc.scalar.activation(
                    p_all[:, : R * cc], sc_ps[:, : R * cc],
                    mybir.ActivationFunctionType.Exp, scale=float(SM_SCALE))

                bc = 0
                for t, s in enumerate(grp):
                    tg = gstart[gi] + t
                    g = geos[s]
                    n8s, nhs = g["nch8"], g["nchh"]
                    nchs = g["nch"]
                    widths = ([min(128, g["coldk"] - 128 * i) for i in range(n8s)]
                              + [min(128, g["hotv"] - 128 * i) for i in range(nhs)])
                    for i in range(nchs):
                        w = widths[i]
                        nc.tensor.matmul(
                            den_ps[:, R * tg: R * (tg + 1)],
                            ones_sb[0:w, :],
                            p_all[0:w, R * (bc + i): R * (bc + i + 1)],
                            start=(i == 0), stop=(i == nchs - 1))
                    bc += nchs

                out_ps = ps_o.tile([HEAD_SIZE, R * len(grp)], f32, tag="ops")
                outtiles.append(out_ps)
                b8 = bh = bc = 0
                for t, s in enumerate(grp):
                    g = geos[s]
                    n8s, nhs = g["nch8"], g["nchh"]
                    for i in range(n8s):
                        w = min(128, g["coldk"] - 128 * i)
                        nc.tensor.matmul(
                            out_ps[:, R * t: R * (t + 1)],
                            v8t[0:w, 128 * (b8 + i): 128 * (b8 + i) + HEAD_SIZE],
                            p_all[0:w, R * (bc + i): R * (bc + i + 1)],
                            start=(i == 0), stop=False)
                    for i in range(nhs):
                        w = min(128, g["hotv"] - 128 * i)
                        nc.tensor.matmul(
                            out_ps[:, R * t: R * (t + 1)],
                            vht[0:w, 128 * (bh + i): 128 * (bh + i) + HEAD_SIZE],
                            p_all[0:w, R * (bc + n8s + i): R * (bc + n8s + i + 1)],
                            start=(n8s + i == 0), stop=(i == nhs - 1))
                    b8 += n8s
                    bh += nhs
                    bc += g["nch"]

            # ---- phase 2: normalization constants (ready mid-kernel) ----
            nc.vector.reciprocal(rden_sb[:], den_ps[:])
            nc.tensor.matmul(rn_ps[:], ones1[:], rden_sb[:],
                             start=True, stop=True)
            nc.vector.tensor_copy(rn_sb[:], rn_ps[:])
            # per-group: scale PSUM accumulators straight into outacc
            for gi, grp in enumerate(groups):
                ng = len(grp)
                cols = slice(R * gstart[gi], R * (gstart[gi] + ng))
                nc.vector.tensor_mul(outacc[:, cols],
                                     outtiles[gi][:, : R * ng],
                                     rn_sb[:, cols])
            nc.sync.dma_start(outD[:], outacc[:])
    nc.finalize()
    return nc


def kernel(q, k_cache, v_cache, block_tables, context_lens, _emulate=False):
    in_maps, geos, order, n8, nh = _build_host_arrays(
        q, k_cache, v_cache, block_tables, context_lens)

    if _emulate:
        outs = [_emulate_core(in_maps[c], geos, order, n8, nh)
                for c in range(N_KV_HEADS)]
    else:
        import os
        from concourse.bass_utils import run_bass_kernel_spmd
        nc = _build_program(geos, order, n8, nh)
        kw = {}
        if os.environ.get("KERNEL_TRACE"):
            kw = dict(trace=True, trace_cores=list(range(8)),
                      tmpdir=os.environ.get("KERNEL_TRACE_DIR") or None)
        try:
            br = run_bass_kernel_spmd(nc, in_maps, list(range(8)), **kw)
        except Exception:
            # transient device errors (e.g. NRT_EXEC_UNIT_UNRECOVERABLE)
            # clear on re-run
            br = run_bass_kernel_spmd(nc, in_maps, list(range(8)), **kw)
        global LAST_EXEC_NS, LAST_RESULTS
        LAST_RESULTS = br
        LAST_EXEC_NS = br.exec_time_ns
        inv = np.empty(NUM_SEQS, np.int64)
        inv[np.asarray(order)] = np.arange(NUM_SEQS)   # original s -> sorted t
        outs = [np.asarray(br.results[c]["out"]).reshape(
            HEAD_SIZE, NUM_SEQS, R).transpose(1, 2, 0)[inv[np.arange(NUM_SEQS)]]
            for c in range(N_KV_HEADS)]

    out = np.zeros((NUM_SEQS, N_Q_HEADS, HEAD_SIZE), np.float32)
    for c in range(N_KV_HEADS):
        out[:, c * R:(c + 1) * R, :] = outs[c]
    return out
